# revision 7
# baseline (speedup 1.0000x reference)
"""Trainium2 Bass kernel: windowed top-k sparse autoencoder.

recon, encoded = kernel(x, W_enc, b_enc, W_dec, b_dec)

  pre     = relu((x - b_dec) @ W_enc.T + b_enc)          (B, T, D)
  ws      = window-sum of pre over WIN=8 frames          (B, T/WIN, D)
  mask    = top-K(ws, K=128) per (batch, window) row
  encoded = pre * mask (broadcast over frames in window) (B, T, D)
  recon   = encoded @ W_dec.T + b_dec                    (B, T, C)

Data-parallel over batch: 16 batches -> 8 NeuronCores, 2 batches each.
Weights replicated per core; no collectives.

All on-device compute is fp32 (top-k boundary gaps on this problem are
~2e-5, far above fp32 noise but far below bf16 noise, so the matmuls must
be true fp32).

Host-side work is layout-only: transposes / reshapes for sharding and for
the PE's (contraction-on-partitions) operand layout.
"""

import sys

sys.path.insert(0, "/opt/trn_rl_repo")

from contextlib import ExitStack

import numpy as np

import concourse.bacc as bacc
import concourse.bass as bass
import concourse.tile as tile
from concourse import mybir
from concourse.bass_utils import run_bass_kernel_spmd

F32 = mybir.dt.float32
AF = mybir.ActivationFunctionType
ALU = mybir.AluOpType
AX = mybir.AxisListType

# Problem shape (hardcoded per contest contract)
B, T, C, D, K, WIN = 16, 512, 1024, 4096, 128, 8
NCORES = 8
BPC = B // NCORES  # batches per core = 2
TP = BPC * T  # frames per core = 1024
NTB = TP // 128  # t-blocks of 128 = 8
NDT = D // 128  # d-tiles = 32
NCT = C // 128  # c-tiles = 8
NW = T // WIN  # windows per batch = 64
ROWS = BPC * NW  # topk rows per core = 128
NCQ = 4  # c-quarters for decode
CQ = C // NCQ  # 256

# set to a BassKernelResults after each kernel() call (for test.py profiling)
LAST_RESULTS = None
TRACE = False
TMPDIR = None


def build_nc():
    nc = bacc.Bacc(
        "TRN2", target_bir_lowering=False, debug=False, num_devices=NCORES
    )

    xT = nc.declare_dram_parameter("xT", [C, TP], F32, isOutput=False)
    We = nc.declare_dram_parameter("We", [NDT, 128, C], F32, isOutput=False)
    Wd = nc.declare_dram_parameter("Wd", [NCQ, NDT, 128, CQ], F32, isOutput=False)
    be_r = nc.declare_dram_parameter("be_r", [128, NDT], F32, isOutput=False)
    bd_r = nc.declare_dram_parameter("bd_r", [128, NCT], F32, isOutput=False)
    bd_bc = nc.declare_dram_parameter("bd_bc", [128, C], F32, isOutput=False)
    ident_d = nc.declare_dram_parameter("ident", [128, 128], F32, isOutput=False)

    enc_out = nc.declare_dram_parameter("enc", [BPC, T, D], F32, isOutput=True)
    rec_out = nc.declare_dram_parameter("rec", [BPC, T, C], F32, isOutput=True)

    with tile.TileContext(nc) as tc, ExitStack() as ctx:
        # ---- long-lived SBUF pools (allocated bottom of stack) ----
        consts = ctx.enter_context(tc.tile_pool(name="consts", bufs=1))
        pre_pool = ctx.enter_context(tc.tile_pool(name="pre", bufs=1))
        ws_pool = ctx.enter_context(tc.tile_pool(name="wsp", bufs=1))
        wst_pool = ctx.enter_context(tc.tile_pool(name="wst", bufs=2))
        mx_pool = ctx.enter_context(tc.tile_pool(name="mx", bufs=2))
        mt_pool = ctx.enter_context(tc.tile_pool(name="mt", bufs=3))
        # psum pool that spans encode + mask transposes
        tp_ps = ctx.enter_context(tc.tile_pool(name="tp_ps", bufs=2, space="PSUM"))

        # consts
        be_sb = consts.tile([128, NDT], F32, tag="be")
        nc.sync.dma_start(be_sb, be_r[:])
        bd_sb = consts.tile([128, NCT], F32, tag="bd")
        nc.sync.dma_start(bd_sb, bd_r[:])
        bdb_sb = consts.tile([128, C], F32, tag="bdb")
        nc.sync.dma_start(bdb_sb, bd_bc[:])
        ident = consts.tile([128, 128], F32, tag="ident")
        nc.sync.dma_start(ident, ident_d[:])

        ws = ws_pool.tile([128, D], F32, tag="ws")

        preT = []
        for j in range(NDT):
            preT.append(pre_pool.tile([128, TP], F32, tag=f"preT{j}", name=f"preT{j}"))

        # ================= Phase E: encode =================
        with ExitStack() as ectx:
            xc_pool = ectx.enter_context(tc.tile_pool(name="xc", bufs=1))
            we_pool = ectx.enter_context(tc.tile_pool(name="we", bufs=2))
            pe_ps = ectx.enter_context(tc.tile_pool(name="pe_ps", bufs=4, space="PSUM"))

            # xcT tiles: (128 c, TP t) per c-tile, minus b_dec (per-partition)
            xcT = []
            for i in range(NCT):
                xct = xc_pool.tile([128, TP], F32, tag=f"xcT{i}", name=f"xcT{i}")
                nc.sync.dma_start(xct, xT[i * 128 : (i + 1) * 128, :])
                nc.vector.tensor_scalar(
                    xct, xct, bd_sb[:, i : i + 1], None, op0=ALU.subtract
                )
                xcT.append(xct)

            for j in range(NDT):
                we_t = we_pool.tile([128, C], F32, tag="we")
                nc.sync.dma_start(we_t, We[j])
                we_v = we_t.rearrange("p (i m) -> p i m", i=NCT)
                pre_j = preT[j]
                for h in range(2):  # halves of TP (512 frames each)
                    ps = pe_ps.tile([128, 512], F32, tag="ps")
                    for i in range(NCT):
                        nc.tensor.matmul(
                            ps,
                            we_v[:, i, :],
                            xcT[i][:, h * 512 : (h + 1) * 512],
                            start=(i == 0),
                            stop=(i == NCT - 1),
                        )
                    # preT = relu(psum + b_enc[dtile]) , fused on ScalarE
                    nc.scalar.activation(
                        pre_j[:, h * 512 : (h + 1) * 512],
                        ps,
                        AF.Relu,
                        bias=be_sb[:, j : j + 1],
                        scale=1.0,
                    )
                # window sums for this d-tile: (128, BPC, NW, WIN) -> (128, BPC, NW)
                wst = wst_pool.tile([128, ROWS], F32, tag="wst")
                nc.vector.tensor_reduce(
                    wst.rearrange("p (b w) -> p b w", b=BPC),
                    pre_j.rearrange("p (b w e) -> p b w e", b=BPC, w=NW, e=WIN),
                    axis=AX.X,
                    op=ALU.add,
                )
                # transpose to row-major ws columns
                pst = tp_ps.tile([128, 128], F32, tag="pst")
                nc.tensor.transpose(pst, wst, ident)
                nc.scalar.copy(ws[:, j * 128 : (j + 1) * 128], pst)

        # ================= top-k peel =================
        # peel top-K values per row (zap to -1); remaining: mask = ws < 0
        for it in range(K // 8):
            mx = mx_pool.tile([128, 8], F32, tag="mx")
            nc.vector.max(out=mx, in_=ws)
            nc.vector.match_replace(
                out=ws, in_to_replace=mx, in_values=ws, imm_value=-1.0
            )
        nc.vector.tensor_scalar(ws, ws, 0.0, None, op0=ALU.is_lt)

        # mask transposes + apply to preT in place (preT becomes encT)
        for j in range(NDT):
            pst = tp_ps.tile([128, 128], F32, tag="pst")
            nc.tensor.transpose(pst, ws[:, j * 128 : (j + 1) * 128], ident)
            mT = mt_pool.tile([128, 128], F32, tag="mT")
            nc.vector.tensor_copy(mT, pst)
            nc.vector.tensor_tensor(
                preT[j].rearrange("p (b w e) -> p b w e", b=BPC, w=NW, e=WIN),
                preT[j].rearrange("p (b w e) -> p b w e", b=BPC, w=NW, e=WIN),
                mT.rearrange("p (b w) -> p b w", b=BPC).to_broadcast(
                    (128, BPC, NW, WIN)
                ),
                op=ALU.mult,
            )

        # ================= Phase D: decode + encoded output =================
        with ExitStack() as dctx:
            wd_pool = dctx.enter_context(tc.tile_pool(name="wd", bufs=3))
            rec_pool = dctx.enter_context(tc.tile_pool(name="rec", bufs=3))
            stg_pool = dctx.enter_context(tc.tile_pool(name="stg", bufs=3))
            psd_pool = dctx.enter_context(tc.tile_pool(name="psd", bufs=3, space="PSUM"))
            eo_ps = dctx.enter_context(tc.tile_pool(name="eo_ps", bufs=2, space="PSUM"))

            # encoded output: transpose encT tiles back to (t, d) and DMA out
            for h8 in range(NTB):
                b = h8 // (NTB // BPC)
                trow = (h8 % (NTB // BPC)) * 128
                for jg in range(NDT // 4):
                    eo = eo_ps.tile([128, 512], F32, tag="eo")
                    for k4 in range(4):
                        j = jg * 4 + k4
                        nc.tensor.transpose(
                            eo[:, k4 * 128 : (k4 + 1) * 128],
                            preT[j][:, h8 * 128 : (h8 + 1) * 128],
                            ident,
                        )
                    stg = stg_pool.tile([128, 512], F32, tag="stg")
                    nc.scalar.copy(stg, eo)
                    nc.sync.dma_start(
                        enc_out[b, trow : trow + 128, jg * 512 : (jg + 1) * 512], stg
                    )

            # decode: recon[t, c] = sum_d encT[d, t] * W_decT[d, c]  (+ b_dec)
            # NOTE: each (q, h8) accumulation group gets its OWN psum tile
            # (bank-padded) — matmul start=True clears has_written bits for
            # the whole bank, so groups must never share a bank.
            for q in range(NCQ):
                wq = []
                for j in range(NDT):
                    wd_t = wd_pool.tile([128, CQ], F32, tag=f"wd{j}", bufs=1)
                    nc.sync.dma_start(wd_t, Wd[q, j])
                    wq.append(wd_t)
                for h8 in range(NTB):
                    b = h8 // (NTB // BPC)
                    trow = (h8 % (NTB // BPC)) * 128
                    psd = psd_pool.tile([128, CQ], F32, tag="psd")
                    for j in range(NDT):
                        nc.tensor.matmul(
                            psd,
                            preT[j][:, h8 * 128 : (h8 + 1) * 128],
                            wq[j],
                            start=(j == 0),
                            stop=(j == NDT - 1),
                        )
                    rsb = rec_pool.tile([128, CQ], F32, tag="rsb")
                    nc.vector.tensor_tensor(
                        rsb,
                        psd,
                        bdb_sb[:, q * CQ : (q + 1) * CQ],
                        op=ALU.add,
                    )
                    nc.sync.dma_start(
                        rec_out[b, trow : trow + 128, q * CQ : (q + 1) * CQ], rsb
                    )

    nc.compile()
    return nc


def _host_prep(x, W_enc, b_enc, W_dec, b_dec):
    x = np.ascontiguousarray(np.asarray(x, dtype=np.float32))
    W_enc = np.asarray(W_enc, dtype=np.float32)
    W_dec = np.asarray(W_dec, dtype=np.float32)
    b_enc = np.asarray(b_enc, dtype=np.float32)
    b_dec = np.asarray(b_dec, dtype=np.float32)

    # W_encT tiles: We[j, p, i*128+m] = W_enc[j*128+m, i*128+p]
    We_h = np.ascontiguousarray(
        W_enc.reshape(NDT, 128, NCT, 128).transpose(0, 3, 2, 1)
    ).reshape(NDT, 128, C)
    # W_decT tiles: Wd[q, j, p, n] = W_dec[q*CQ+n, j*128+p]
    Wd_h = np.ascontiguousarray(
        W_dec.T.reshape(NDT, 128, NCQ, CQ).transpose(2, 0, 1, 3)
    )
    be_r = np.ascontiguousarray(b_enc.reshape(NDT, 128).T)
    bd_r = np.ascontiguousarray(b_dec.reshape(NCT, 128).T)
    bd_bc = np.ascontiguousarray(np.broadcast_to(b_dec[None, :], (128, C)))
    ident = np.eye(128, dtype=np.float32)

    in_maps = []
    for c in range(NCORES):
        xs = x[c * BPC : (c + 1) * BPC]  # (BPC, T, C)
        xT_c = np.ascontiguousarray(xs.transpose(2, 0, 1).reshape(C, TP))
        in_maps.append(
            {
                "xT": xT_c,
                "We": We_h,
                "Wd": Wd_h,
                "be_r": be_r,
                "bd_r": bd_r,
                "bd_bc": bd_bc,
                "ident": ident,
            }
        )
    return in_maps


def kernel(x, W_enc, b_enc, W_dec, b_dec):
    global LAST_RESULTS
    in_maps = _host_prep(x, W_enc, b_enc, W_dec, b_dec)
    nc = build_nc()
    kw = {}
    if TRACE:
        kw["trace"] = True
        if TMPDIR:
            kw["tmpdir"] = TMPDIR
    res = run_bass_kernel_spmd(nc, in_maps, list(range(NCORES)), **kw)
    LAST_RESULTS = res
    recon = np.concatenate([res.results[c]["rec"] for c in range(NCORES)], axis=0)
    encoded = np.concatenate([res.results[c]["enc"] for c in range(NCORES)], axis=0)
    return recon, encoded


# revision 8
# speedup vs baseline: 1.0189x; 1.0189x over previous
"""Trainium2 Bass kernel: windowed top-k sparse autoencoder.

recon, encoded = kernel(x, W_enc, b_enc, W_dec, b_dec)

  pre     = relu((x - b_dec) @ W_enc.T + b_enc)          (B, T, D)
  ws      = window-sum of pre over WIN=8 frames          (B, T/WIN, D)
  mask    = top-K(ws, K=128) per (batch, window) row
  encoded = pre * mask (broadcast over frames in window) (B, T, D)
  recon   = encoded @ W_dec.T + b_dec                    (B, T, C)

Data-parallel over batch: 16 batches -> 8 NeuronCores, 2 batches each.
Weights replicated per core; no collectives.

All on-device compute is fp32 (top-k boundary gaps on this problem are
~2e-5 — far above fp32 noise but far below bf16/tf32 noise, so the encode
matmul must be true fp32).

Host-side work is layout-only: transposes / reshapes for sharding and for
the PE's (contraction-on-partitions) operand layout.

Kernel structure (per core; activations kept transposed as preT (d, t)):
  encode: stream W_encT by d-tile; PE matmul (c-tiles on partitions) ->
          psum; ScalarE fused relu+bias -> preT; DVE window-sum reduce;
          small PE transpose lands ws rows (128 topk rows x 4096 d).
  top-k:  pipelined under encode — every 4 d-tiles, DVE merges the new
          512 ws columns into a running top-128 via 16 x (max8 +
          match_replace).  thr = min(top128); mask = (ws >= thr), exact
          (no fp32 ties; verified on this problem's data).
  apply:  transpose mask tiles, broadcast-multiply into preT (= encT).
  out:    PE transposes encT tiles back to (t, d) for the encoded output.
  decode: stream W_decT; PE accumulates over all 32 d-tiles into 8
          bank-padded psum tiles (one per t-block; never share banks
          between accumulation groups); DVE adds b_dec; DMA out.
"""

import sys

sys.path.insert(0, "/opt/trn_rl_repo")

from contextlib import ExitStack

import numpy as np

import concourse.bacc as bacc
import concourse.bass as bass
import concourse.tile as tile
from concourse import mybir
from concourse.bass_utils import run_bass_kernel_spmd

F32 = mybir.dt.float32
F32R = mybir.dt.float32r
AF = mybir.ActivationFunctionType
ALU = mybir.AluOpType
AX = mybir.AxisListType

# Problem shape (hardcoded per contest contract)
B, T, C, D, K, WIN = 16, 512, 1024, 4096, 128, 8
NCORES = 8
BPC = B // NCORES  # batches per core = 2
TP = BPC * T  # frames per core = 1024
NTB = TP // 128  # t-blocks of 128 = 8
NDT = D // 128  # d-tiles = 32
NCT = C // 128  # c-tiles = 8
NW = T // WIN  # windows per batch = 64
ROWS = BPC * NW  # topk rows per core = 128
NCQ = 4  # c-quarters for decode
CQ = C // NCQ  # 256
MERGE = 4  # d-tiles per topk merge step
NMERGE = NDT // MERGE

# set to a BassKernelResults after each kernel() call (for test.py profiling)
LAST_RESULTS = None
TRACE = False
TMPDIR = None
DECODE_F32R = False  # use float32r for the decode matmul (recon only)


def build_nc():
    nc = bacc.Bacc(
        "TRN2", target_bir_lowering=False, debug=False, num_devices=NCORES
    )

    xT = nc.declare_dram_parameter("xT", [C, TP], F32, isOutput=False)
    We = nc.declare_dram_parameter("We", [NDT, 128, C], F32, isOutput=False)
    Wd = nc.declare_dram_parameter("Wd", [NCQ, NDT, 128, CQ], F32, isOutput=False)
    be_r = nc.declare_dram_parameter("be_r", [128, NDT], F32, isOutput=False)
    bd_r = nc.declare_dram_parameter("bd_r", [128, NCT], F32, isOutput=False)
    bd_bc = nc.declare_dram_parameter("bd_bc", [128, C], F32, isOutput=False)
    ident_d = nc.declare_dram_parameter("ident", [128, 128], F32, isOutput=False)

    enc_out = nc.declare_dram_parameter("enc", [BPC, T, D], F32, isOutput=True)
    rec_out = nc.declare_dram_parameter("rec", [BPC, T, C], F32, isOutput=True)

    with tile.TileContext(nc) as tc, ExitStack() as ctx:
        # ---- whole-kernel SBUF pools ----
        consts = ctx.enter_context(tc.tile_pool(name="consts", bufs=1))
        pre_pool = ctx.enter_context(tc.tile_pool(name="pre", bufs=1))

        be_sb = consts.tile([128, NDT], F32, tag="be")
        nc.sync.dma_start(be_sb, be_r[:])
        bd_sb = consts.tile([128, NCT], F32, tag="bd")
        nc.sync.dma_start(bd_sb, bd_r[:])
        bdb_sb = consts.tile([128, C], F32, tag="bdb")
        nc.sync.dma_start(bdb_sb, bd_bc[:])
        ident = consts.tile([128, 128], F32, tag="ident")
        nc.sync.dma_start(ident, ident_d[:])

        preT = []
        for j in range(NDT):
            preT.append(pre_pool.tile([128, TP], F32, tag=f"preT{j}", name=f"preT{j}"))

        # ---- pools that live encode -> mask application ----
        tctx = ExitStack()
        ws_pool = tctx.enter_context(tc.tile_pool(name="wsp", bufs=1))
        cand_pool = tctx.enter_context(tc.tile_pool(name="cand", bufs=2))
        thr_pool = tctx.enter_context(tc.tile_pool(name="thr", bufs=1))
        mt_pool = tctx.enter_context(tc.tile_pool(name="mt", bufs=3))
        tp_ps = tctx.enter_context(tc.tile_pool(name="tp_ps", bufs=2, space="PSUM"))

        ws = ws_pool.tile([128, D], F32, tag="ws")

        cands = [
            cand_pool.tile(
                [128, K + MERGE * 128], F32, tag=f"cand{g % 2}", name=f"cand{g}"
            )
            for g in range(NMERGE + 1)
        ]
        nc.vector.memset(cands[0][:, 0:K], -1.0)

        # ================= Phase E: encode =================
        with ExitStack() as ectx:
            xc_pool = ectx.enter_context(tc.tile_pool(name="xc", bufs=1))
            we_pool = ectx.enter_context(tc.tile_pool(name="we", bufs=2))
            pe_ps = ectx.enter_context(tc.tile_pool(name="pe_ps", bufs=4, space="PSUM"))

            # xcT tiles: (128 c, TP t) per c-tile, minus b_dec (per-partition)
            xcT = []
            for i in range(NCT):
                xct = xc_pool.tile([128, TP], F32, tag=f"xcT{i}", name=f"xcT{i}")
                nc.sync.dma_start(xct, xT[i * 128 : (i + 1) * 128, :])
                nc.vector.tensor_scalar(
                    xct, xct, bd_sb[:, i : i + 1], None, op0=ALU.subtract
                )
                xcT.append(xct)

            for j in range(NDT):
                we_t = we_pool.tile([128, C], F32, tag="we")
                nc.sync.dma_start(we_t, We[j])
                we_v = we_t.rearrange("p (i m) -> p i m", i=NCT)
                pre_j = preT[j]
                for h in range(2):  # halves of TP (512 frames each)
                    ps = pe_ps.tile([128, 512], F32, tag="ps")
                    for i in range(NCT):
                        nc.tensor.matmul(
                            ps,
                            we_v[:, i, :],
                            xcT[i][:, h * 512 : (h + 1) * 512],
                            start=(i == 0),
                            stop=(i == NCT - 1),
                        )
                    # preT = relu(psum + b_enc[dtile]) , fused on ScalarE
                    nc.scalar.activation(
                        pre_j[:, h * 512 : (h + 1) * 512],
                        ps,
                        AF.Relu,
                        bias=be_sb[:, j : j + 1],
                        scale=1.0,
                    )
                # window sums for this d-tile: (128, BPC, NW, WIN) -> (128, BPC*NW)
                wst = mt_pool.tile([128, ROWS], F32, tag="wst")
                nc.vector.tensor_reduce(
                    wst.rearrange("p (b w) -> p b w", b=BPC),
                    pre_j.rearrange("p (b w e) -> p b w e", b=BPC, w=NW, e=WIN),
                    axis=AX.X,
                    op=ALU.add,
                )
                # transpose to row-major ws columns
                pst = tp_ps.tile([128, 128], F32, tag="pst")
                nc.tensor.transpose(pst, wst, ident)
                nc.scalar.copy(ws[:, j * 128 : (j + 1) * 128], pst)

                # pipelined top-k: merge the last MERGE d-tiles' ws columns
                # into the running top-K (kept in cands[g][:, 0:K])
                if (j + 1) % MERGE == 0:
                    g = (j + 1) // MERGE - 1
                    src, dst = cands[g], cands[g + 1]
                    nc.vector.tensor_copy(
                        src[:, K:], ws[:, g * MERGE * 128 : (g + 1) * MERGE * 128]
                    )
                    for it in range(K // 8):
                        nc.vector.max(out=dst[:, it * 8 : (it + 1) * 8], in_=src)
                        nc.vector.match_replace(
                            out=src,
                            in_to_replace=dst[:, it * 8 : (it + 1) * 8],
                            in_values=src,
                            imm_value=-1.0,
                        )

        # ================= threshold + mask + apply =================
        thr = thr_pool.tile([128, 1], F32, tag="thr")
        nc.vector.tensor_reduce(
            thr, cands[NMERGE][:, 0:K], axis=AX.X, op=ALU.min
        )
        # mask in place: ws = (ws >= thr)  — exact top-K (no fp32 ties here)
        nc.vector.tensor_scalar(ws, ws, thr, None, op0=ALU.is_ge)

        # mask transposes + apply to preT in place (preT becomes encT)
        for j in range(NDT):
            pst = tp_ps.tile([128, 128], F32, tag="pst")
            nc.tensor.transpose(pst, ws[:, j * 128 : (j + 1) * 128], ident)
            mT = mt_pool.tile([128, 128], F32, tag="mT")
            nc.vector.tensor_copy(mT, pst)
            nc.vector.tensor_tensor(
                preT[j].rearrange("p (b w e) -> p b w e", b=BPC, w=NW, e=WIN),
                preT[j].rearrange("p (b w e) -> p b w e", b=BPC, w=NW, e=WIN),
                mT.rearrange("p (b w) -> p b w", b=BPC).to_broadcast(
                    (128, BPC, NW, WIN)
                ),
                op=ALU.mult,
            )
        tctx.close()  # frees ws/cand/mt SBUF and tp_ps PSUM

        # ================= Phase D: encoded output + decode =================
        with ExitStack() as dctx:
            stg_pool = dctx.enter_context(tc.tile_pool(name="stg", bufs=3))
            wd_pool = dctx.enter_context(tc.tile_pool(name="wd", bufs=4))
            rec_pool = dctx.enter_context(tc.tile_pool(name="rec", bufs=3))

            # encoded output: transpose encT tiles back to (t, d), DMA out
            with ExitStack() as ectx2:
                eo_ps = ectx2.enter_context(
                    tc.tile_pool(name="eo_ps", bufs=4, space="PSUM")
                )
                for h8 in range(NTB):
                    b = h8 // (NTB // BPC)
                    trow = (h8 % (NTB // BPC)) * 128
                    for jg in range(NDT // 4):
                        eo = eo_ps.tile([128, 512], F32, tag="eo")
                        for k4 in range(4):
                            j = jg * 4 + k4
                            nc.tensor.transpose(
                                eo[:, k4 * 128 : (k4 + 1) * 128],
                                preT[j][:, h8 * 128 : (h8 + 1) * 128],
                                ident,
                            )
                        stg = stg_pool.tile([128, 512], F32, tag="stg")
                        nc.scalar.copy(stg, eo)
                        nc.sync.dma_start(
                            enc_out[b, trow : trow + 128, jg * 512 : (jg + 1) * 512],
                            stg,
                        )

            # decode: recon[t, c] = sum_d encT[d, t] * W_decT[d, c]  (+ b_dec)
            # 8 bank-padded psum tiles (one per t-block) — accumulation groups
            # must never share a PSUM bank (start=True clears the whole bank's
            # has_written bits).
            psd_pool = dctx.enter_context(
                tc.tile_pool(name="psd", bufs=1, space="PSUM")
            )
            mmdt = F32R if DECODE_F32R else F32
            for q in range(NCQ):
                psds = [
                    psd_pool.tile([128, CQ], F32, tag=f"psd{h8}", name=f"psd{h8}_{q}")
                    for h8 in range(NTB)
                ]
                for j in range(NDT):
                    wd_t = wd_pool.tile([128, CQ], F32, tag="wd")
                    nc.sync.dma_start(wd_t, Wd[q, j])
                    for h8 in range(NTB):
                        nc.tensor.matmul(
                            psds[h8],
                            preT[j][:, h8 * 128 : (h8 + 1) * 128].bitcast(mmdt),
                            wd_t[:].bitcast(mmdt),
                            start=(j == 0),
                            stop=(j == NDT - 1),
                        )
                for h8 in range(NTB):
                    b = h8 // (NTB // BPC)
                    trow = (h8 % (NTB // BPC)) * 128
                    rsb = rec_pool.tile([128, CQ], F32, tag="rsb")
                    nc.vector.tensor_tensor(
                        rsb,
                        psds[h8],
                        bdb_sb[:, q * CQ : (q + 1) * CQ],
                        op=ALU.add,
                    )
                    nc.sync.dma_start(
                        rec_out[b, trow : trow + 128, q * CQ : (q + 1) * CQ], rsb
                    )

    nc.compile()
    return nc


def _host_prep(x, W_enc, b_enc, W_dec, b_dec):
    x = np.ascontiguousarray(np.asarray(x, dtype=np.float32))
    W_enc = np.asarray(W_enc, dtype=np.float32)
    W_dec = np.asarray(W_dec, dtype=np.float32)
    b_enc = np.asarray(b_enc, dtype=np.float32)
    b_dec = np.asarray(b_dec, dtype=np.float32)

    # W_encT tiles: We[j, p, i*128+m] = W_enc[j*128+m, i*128+p]
    We_h = np.ascontiguousarray(
        W_enc.reshape(NDT, 128, NCT, 128).transpose(0, 3, 2, 1)
    ).reshape(NDT, 128, C)
    # W_decT tiles: Wd[q, j, p, n] = W_dec[q*CQ+n, j*128+p]
    Wd_h = np.ascontiguousarray(
        W_dec.T.reshape(NDT, 128, NCQ, CQ).transpose(2, 0, 1, 3)
    )
    be_r = np.ascontiguousarray(b_enc.reshape(NDT, 128).T)
    bd_r = np.ascontiguousarray(b_dec.reshape(NCT, 128).T)
    bd_bc = np.ascontiguousarray(np.broadcast_to(b_dec[None, :], (128, C)))
    ident = np.eye(128, dtype=np.float32)

    in_maps = []
    for c in range(NCORES):
        xs = x[c * BPC : (c + 1) * BPC]  # (BPC, T, C)
        xT_c = np.ascontiguousarray(xs.transpose(2, 0, 1).reshape(C, TP))
        in_maps.append(
            {
                "xT": xT_c,
                "We": We_h,
                "Wd": Wd_h,
                "be_r": be_r,
                "bd_r": bd_r,
                "bd_bc": bd_bc,
                "ident": ident,
            }
        )
    return in_maps


def kernel(x, W_enc, b_enc, W_dec, b_dec):
    global LAST_RESULTS
    in_maps = _host_prep(x, W_enc, b_enc, W_dec, b_dec)
    nc = build_nc()
    kw = {}
    if TRACE:
        kw["trace"] = True
        if TMPDIR:
            kw["tmpdir"] = TMPDIR
    res = run_bass_kernel_spmd(nc, in_maps, list(range(NCORES)), **kw)
    LAST_RESULTS = res
    recon = np.concatenate([res.results[c]["rec"] for c in range(NCORES)], axis=0)
    encoded = np.concatenate([res.results[c]["enc"] for c in range(NCORES)], axis=0)
    return recon, encoded


# revision 13
# speedup vs baseline: 1.0222x; 1.0032x over previous
"""Trainium2 Bass kernel: windowed top-k sparse autoencoder.

recon, encoded = kernel(x, W_enc, b_enc, W_dec, b_dec)

  pre     = relu((x - b_dec) @ W_enc.T + b_enc)          (B, T, D)
  ws      = window-sum of pre over WIN=8 frames          (B, T/WIN, D)
  mask    = top-K(ws, K=128) per (batch, window) row
  encoded = pre * mask (broadcast over frames in window) (B, T, D)
  recon   = encoded @ W_dec.T + b_dec                    (B, T, C)

Data-parallel over batch: 16 batches -> 8 NeuronCores, 2 batches each.
Weights replicated per core; no collectives.

All on-device compute is fp32 (top-k boundary gaps on this problem are
~2e-5 — far above fp32 noise but far below bf16/tf32 noise, so the encode
matmul must be true fp32).

Host-side work is layout-only: transposes / reshapes for sharding and for
the PE's (contraction-on-partitions) operand layout.

Kernel structure (per core; activations kept transposed as preT (d, t)):
  encode: stream W_encT by d-tile; PE matmul (c-tiles on partitions) ->
          psum; ScalarE fused relu+bias -> preT; DVE window-sum reduce;
          small PE transpose lands ws rows (128 topk rows x 4096 d).
  top-k:  pipelined under encode — every 4 d-tiles, DVE merges the new
          512 ws columns into a running top-128 via 16 x (max8 +
          match_replace).  thr = min(top128); mask = (ws >= thr), exact
          (no fp32 ties; verified on this problem's data).
  apply:  transpose mask tiles, broadcast-multiply into preT (= encT).
  out:    PE transposes encT tiles back to (t, d) for the encoded output.
  decode: stream W_decT; PE accumulates over all 32 d-tiles into 8
          bank-padded psum tiles (one per t-block; never share banks
          between accumulation groups); DVE adds b_dec; DMA out.
"""

import sys

sys.path.insert(0, "/opt/trn_rl_repo")

from contextlib import ExitStack

import numpy as np

import concourse.bacc as bacc
import concourse.bass as bass
import concourse.tile as tile
from concourse import mybir
from concourse.bass_utils import run_bass_kernel_spmd

F32 = mybir.dt.float32
F32R = mybir.dt.float32r
AF = mybir.ActivationFunctionType
ALU = mybir.AluOpType
AX = mybir.AxisListType

# Problem shape (hardcoded per contest contract)
B, T, C, D, K, WIN = 16, 512, 1024, 4096, 128, 8
NCORES = 8
BPC = B // NCORES  # batches per core = 2
TP = BPC * T  # frames per core = 1024
NTB = TP // 128  # t-blocks of 128 = 8
NDT = D // 128  # d-tiles = 32
NCT = C // 128  # c-tiles = 8
NW = T // WIN  # windows per batch = 64
ROWS = BPC * NW  # topk rows per core = 128
NCQ = 4  # c-quarters for decode
CQ = C // NCQ  # 256
MERGE = 4  # d-tiles per topk merge step
NMERGE = NDT // MERGE

# set to a BassKernelResults after each kernel() call (for test.py profiling)
LAST_RESULTS = None
TRACE = False
TMPDIR = None
DECODE_F32R = False  # use float32r for the decode matmul (recon only)


def build_nc():
    nc = bacc.Bacc(
        "TRN2", target_bir_lowering=False, debug=False, num_devices=NCORES
    )

    xT = nc.declare_dram_parameter("xT", [C, TP], F32, isOutput=False)
    We = nc.declare_dram_parameter("We", [NDT, 128, C], F32, isOutput=False)
    Wd = nc.declare_dram_parameter("Wd", [NCQ, NDT, 128, CQ], F32, isOutput=False)
    be_r = nc.declare_dram_parameter("be_r", [128, NDT], F32, isOutput=False)
    bd_r = nc.declare_dram_parameter("bd_r", [128, NCT], F32, isOutput=False)
    bd_bc = nc.declare_dram_parameter("bd_bc", [128, C], F32, isOutput=False)
    ident_d = nc.declare_dram_parameter("ident", [128, 128], F32, isOutput=False)

    enc_out = nc.declare_dram_parameter("enc", [BPC, T, D], F32, isOutput=True)
    rec_out = nc.declare_dram_parameter("rec", [BPC, T, C], F32, isOutput=True)

    with tile.TileContext(nc) as tc, ExitStack() as ctx:
        # ---- whole-kernel SBUF pools ----
        consts = ctx.enter_context(tc.tile_pool(name="consts", bufs=1))
        pre_pool = ctx.enter_context(tc.tile_pool(name="pre", bufs=1))

        be_sb = consts.tile([128, NDT], F32, tag="be")
        nc.sync.dma_start(be_sb, be_r[:])
        bd_sb = consts.tile([128, NCT], F32, tag="bd")
        nc.sync.dma_start(bd_sb, bd_r[:])
        bdb_sb = consts.tile([128, C], F32, tag="bdb")
        nc.sync.dma_start(bdb_sb, bd_bc[:])
        ident = consts.tile([128, 128], F32, tag="ident")
        nc.sync.dma_start(ident, ident_d[:])

        preT = []
        for j in range(NDT):
            preT.append(pre_pool.tile([128, TP], F32, tag=f"preT{j}", name=f"preT{j}"))

        # ---- pools that live encode -> mask application ----
        tctx = ExitStack()
        ws_pool = tctx.enter_context(tc.tile_pool(name="wsp", bufs=1))
        cand_pool = tctx.enter_context(tc.tile_pool(name="cand", bufs=2))
        thr_pool = tctx.enter_context(tc.tile_pool(name="thr", bufs=1))
        mt_pool = tctx.enter_context(tc.tile_pool(name="mt", bufs=3))
        tp_ps = tctx.enter_context(tc.tile_pool(name="tp_ps", bufs=2, space="PSUM"))

        ws = ws_pool.tile([128, D], F32, tag="ws")

        # merge schedule: after these d-tile counts, fold the new ws columns
        # into the running top-K.  Last two steps are small so the tail of the
        # chain (which serializes with decode start) is short.
        merge_pts = [4, 8, 12, 16, 20, 24, 28, 30, 32]
        cands = [
            cand_pool.tile(
                [128, K + MERGE * 128], F32, tag=f"cand{g % 2}", name=f"cand{g}"
            )
            for g in range(len(merge_pts) + 1)
        ]
        nc.vector.memset(cands[0][:, 0:K], -1.0)

        # ================= Phase E: encode =================
        with ExitStack() as ectx:
            xc_pool = ectx.enter_context(tc.tile_pool(name="xc", bufs=1))
            we_pool = ectx.enter_context(tc.tile_pool(name="we", bufs=2))
            pe_ps = ectx.enter_context(tc.tile_pool(name="pe_ps", bufs=4, space="PSUM"))

            # xcT tiles: (128 c, TP t) per c-tile, minus b_dec (per-partition)
            xcT = []
            for i in range(NCT):
                xct = xc_pool.tile([128, TP], F32, tag=f"xcT{i}", name=f"xcT{i}")
                xcT.append(xct)
            # split halves so the first encode matmuls (t-chunk 0) can start
            # before the whole 4MB of x has landed
            for h in range(2):
                sl = slice(h * 512, (h + 1) * 512)
                for i in range(NCT):
                    nc.sync.dma_start(xcT[i][:, sl], xT[i * 128 : (i + 1) * 128, sl])
                    nc.vector.tensor_scalar(
                        xcT[i][:, sl], xcT[i][:, sl], bd_sb[:, i : i + 1], None,
                        op0=ALU.subtract,
                    )

            for j in range(NDT):
                we_t = we_pool.tile([128, C], F32, tag="we")
                nc.sync.dma_start(we_t, We[j])
                we_v = we_t.rearrange("p (i m) -> p i m", i=NCT)
                pre_j = preT[j]
                for h in range(2):  # halves of TP (512 frames each)
                    ps = pe_ps.tile([128, 512], F32, tag="ps")
                    for i in range(NCT):
                        nc.tensor.matmul(
                            ps,
                            we_v[:, i, :],
                            xcT[i][:, h * 512 : (h + 1) * 512],
                            start=(i == 0),
                            stop=(i == NCT - 1),
                        )
                    # preT = relu(psum + b_enc[dtile]) , fused on ScalarE
                    nc.scalar.activation(
                        pre_j[:, h * 512 : (h + 1) * 512],
                        ps,
                        AF.Relu,
                        bias=be_sb[:, j : j + 1],
                        scale=1.0,
                    )
                # window sums for this d-tile: (128, BPC, NW, WIN) -> (128, BPC*NW)
                wst = mt_pool.tile([128, ROWS], F32, tag="wst")
                nc.vector.tensor_reduce(
                    wst.rearrange("p (b w) -> p b w", b=BPC),
                    pre_j.rearrange("p (b w e) -> p b w e", b=BPC, w=NW, e=WIN),
                    axis=AX.X,
                    op=ALU.add,
                )
                # transpose to row-major ws columns
                pst = tp_ps.tile([128, 128], F32, tag="pst")
                nc.tensor.transpose(pst, wst, ident)
                nc.scalar.copy(ws[:, j * 128 : (j + 1) * 128], pst)

                # pipelined top-k: merge the newest ws columns into the
                # running top-K (kept in cands[g][:, 0:K])
                if (j + 1) in merge_pts:
                    g = merge_pts.index(j + 1)
                    lo = 0 if g == 0 else merge_pts[g - 1]
                    src, dst = cands[g], cands[g + 1]
                    width = (j + 1 - lo) * 128
                    nc.vector.tensor_copy(
                        src[:, K : K + width], ws[:, lo * 128 : (j + 1) * 128]
                    )
                    for it in range(K // 8):
                        nc.vector.max(
                            out=dst[:, it * 8 : (it + 1) * 8],
                            in_=src[:, 0 : K + width],
                        )
                        nc.vector.match_replace(
                            out=src[:, 0 : K + width],
                            in_to_replace=dst[:, it * 8 : (it + 1) * 8],
                            in_values=src[:, 0 : K + width],
                            imm_value=-1.0,
                        )

        # ================= threshold + mask + apply =================
        thr = thr_pool.tile([128, 1], F32, tag="thr")
        nc.vector.tensor_reduce(
            thr, cands[len(merge_pts)][:, 0:K], axis=AX.X, op=ALU.min
        )
        # mask in place: ws = (ws >= thr)  — exact top-K (no fp32 ties here)
        nc.vector.tensor_scalar(ws, ws, thr, None, op0=ALU.is_ge)

        # mask transposes + apply to preT in place (preT becomes encT)
        for j in range(NDT):
            pst = tp_ps.tile([128, 128], F32, tag="pst")
            nc.tensor.transpose(pst, ws[:, j * 128 : (j + 1) * 128], ident)
            mT = mt_pool.tile([128, 128], F32, tag="mT")
            nc.vector.tensor_copy(mT, pst)
            nc.vector.tensor_tensor(
                preT[j].rearrange("p (b w e) -> p b w e", b=BPC, w=NW, e=WIN),
                preT[j].rearrange("p (b w e) -> p b w e", b=BPC, w=NW, e=WIN),
                mT.rearrange("p (b w) -> p b w", b=BPC).to_broadcast(
                    (128, BPC, NW, WIN)
                ),
                op=ALU.mult,
            )
        tctx.close()  # frees ws/cand/mt SBUF and tp_ps PSUM

        # ================= Phase D: encoded output + decode =================
        with ExitStack() as dctx:
            stg_pool = dctx.enter_context(tc.tile_pool(name="stg", bufs=3))
            wd_pool = dctx.enter_context(tc.tile_pool(name="wd", bufs=4))
            rec_pool = dctx.enter_context(tc.tile_pool(name="rec", bufs=3))

            # encoded output: transpose encT tiles back to (t, d), DMA out
            with ExitStack() as ectx2:
                eo_ps = ectx2.enter_context(
                    tc.tile_pool(name="eo_ps", bufs=4, space="PSUM")
                )
                for h8 in range(NTB):
                    b = h8 // (NTB // BPC)
                    trow = (h8 % (NTB // BPC)) * 128
                    for jg in range(NDT // 4):
                        eo = eo_ps.tile([128, 512], F32, tag="eo")
                        for k4 in range(4):
                            j = jg * 4 + k4
                            nc.tensor.transpose(
                                eo[:, k4 * 128 : (k4 + 1) * 128],
                                preT[j][:, h8 * 128 : (h8 + 1) * 128],
                                ident,
                            )
                        stg = stg_pool.tile([128, 512], F32, tag="stg")
                        nc.scalar.copy(stg, eo)
                        nc.sync.dma_start(
                            enc_out[b, trow : trow + 128, jg * 512 : (jg + 1) * 512],
                            stg,
                        )

            # decode: recon[t, c] = sum_d encT[d, t] * W_decT[d, c]  (+ b_dec)
            # 8 bank-padded psum tiles (one per t-block) — accumulation groups
            # must never share a PSUM bank (start=True clears the whole bank's
            # has_written bits).
            psd_pool = dctx.enter_context(
                tc.tile_pool(name="psd", bufs=1, space="PSUM")
            )
            mmdt = F32R if DECODE_F32R else F32
            for q in range(NCQ):
                psds = [
                    psd_pool.tile([128, CQ], F32, tag=f"psd{h8}", name=f"psd{h8}_{q}")
                    for h8 in range(NTB)
                ]
                for j in range(NDT):
                    wd_t = wd_pool.tile([128, CQ], F32, tag="wd")
                    nc.sync.dma_start(wd_t, Wd[q, j])
                    for h8 in range(NTB):
                        nc.tensor.matmul(
                            psds[h8],
                            preT[j][:, h8 * 128 : (h8 + 1) * 128].bitcast(mmdt),
                            wd_t[:].bitcast(mmdt),
                            start=(j == 0),
                            stop=(j == NDT - 1),
                        )
                for h8 in range(NTB):
                    b = h8 // (NTB // BPC)
                    trow = (h8 % (NTB // BPC)) * 128
                    rsb = rec_pool.tile([128, CQ], F32, tag="rsb")
                    nc.vector.tensor_tensor(
                        rsb,
                        psds[h8],
                        bdb_sb[:, q * CQ : (q + 1) * CQ],
                        op=ALU.add,
                    )
                    nc.sync.dma_start(
                        rec_out[b, trow : trow + 128, q * CQ : (q + 1) * CQ], rsb
                    )

    nc.compile()
    return nc


def _host_prep(x, W_enc, b_enc, W_dec, b_dec):
    x = np.ascontiguousarray(np.asarray(x, dtype=np.float32))
    W_enc = np.asarray(W_enc, dtype=np.float32)
    W_dec = np.asarray(W_dec, dtype=np.float32)
    b_enc = np.asarray(b_enc, dtype=np.float32)
    b_dec = np.asarray(b_dec, dtype=np.float32)

    # W_encT tiles: We[j, p, i*128+m] = W_enc[j*128+m, i*128+p]
    We_h = np.ascontiguousarray(
        W_enc.reshape(NDT, 128, NCT, 128).transpose(0, 3, 2, 1)
    ).reshape(NDT, 128, C)
    # W_decT tiles: Wd[q, j, p, n] = W_dec[q*CQ+n, j*128+p]
    Wd_h = np.ascontiguousarray(
        W_dec.T.reshape(NDT, 128, NCQ, CQ).transpose(2, 0, 1, 3)
    )
    be_r = np.ascontiguousarray(b_enc.reshape(NDT, 128).T)
    bd_r = np.ascontiguousarray(b_dec.reshape(NCT, 128).T)
    bd_bc = np.ascontiguousarray(np.broadcast_to(b_dec[None, :], (128, C)))
    ident = np.eye(128, dtype=np.float32)

    in_maps = []
    for c in range(NCORES):
        xs = x[c * BPC : (c + 1) * BPC]  # (BPC, T, C)
        xT_c = np.ascontiguousarray(xs.transpose(2, 0, 1).reshape(C, TP))
        in_maps.append(
            {
                "xT": xT_c,
                "We": We_h,
                "Wd": Wd_h,
                "be_r": be_r,
                "bd_r": bd_r,
                "bd_bc": bd_bc,
                "ident": ident,
            }
        )
    return in_maps


def kernel(x, W_enc, b_enc, W_dec, b_dec):
    global LAST_RESULTS
    in_maps = _host_prep(x, W_enc, b_enc, W_dec, b_dec)
    nc = build_nc()
    kw = {}
    if TRACE:
        kw["trace"] = True
        if TMPDIR:
            kw["tmpdir"] = TMPDIR
    res = run_bass_kernel_spmd(nc, in_maps, list(range(NCORES)), **kw)
    LAST_RESULTS = res
    recon = np.concatenate([res.results[c]["rec"] for c in range(NCORES)], axis=0)
    encoded = np.concatenate([res.results[c]["enc"] for c in range(NCORES)], axis=0)
    return recon, encoded


# revision 18
# speedup vs baseline: 1.4310x; 1.4000x over previous
"""Trainium2 Bass kernel: windowed top-k sparse autoencoder.

recon, encoded = kernel(x, W_enc, b_enc, W_dec, b_dec)

  pre     = relu((x - b_dec) @ W_enc.T + b_enc)          (B, T, D)
  ws      = window-sum of pre over WIN=8 frames          (B, T/WIN, D)
  mask    = top-K(ws, K=128) per (batch, window) row
  encoded = pre * mask (broadcast over frames in window) (B, T, D)
  recon   = encoded @ W_dec.T + b_dec                    (B, T, C)

Data-parallel over batch: 16 batches -> 8 NeuronCores, 2 batches each.
Weights replicated per core; no collectives.

All on-device compute is fp32 (top-k boundary gaps on this problem are
~2e-5 — far above fp32 noise but far below bf16/tf32 noise, so the encode
matmul must be true fp32).

Host-side work is layout-only: transposes / reshapes for sharding and for
the PE's (contraction-on-partitions) operand layout.

Kernel structure (per core; activations kept transposed as preT (d, t)):
  encode: stream W_encT by d-tile; PE matmul (c-tiles on partitions) ->
          psum; ScalarE fused relu+bias -> preT; DVE window-sum reduce;
          small PE transpose lands ws rows (128 topk rows x 4096 d).
  top-k:  pipelined under encode — every 4 d-tiles, DVE merges the new
          512 ws columns into a running top-128 via 16 x (max8 +
          match_replace).  thr = min(top128); mask = (ws >= thr), exact
          (no fp32 ties; verified on this problem's data).
  apply:  transpose mask tiles, broadcast-multiply into preT (= encT).
  out:    PE transposes encT tiles back to (t, d) for the encoded output.
  decode: stream W_decT; PE accumulates over all 32 d-tiles into 8
          bank-padded psum tiles (one per t-block; never share banks
          between accumulation groups); DVE adds b_dec; DMA out.
"""

import sys

sys.path.insert(0, "/opt/trn_rl_repo")

from contextlib import ExitStack

import numpy as np

import concourse.bacc as bacc
import concourse.bass as bass
import concourse.tile as tile
from concourse import mybir
from concourse.bass_utils import run_bass_kernel_spmd

F32 = mybir.dt.float32
F32R = mybir.dt.float32r
AF = mybir.ActivationFunctionType
ALU = mybir.AluOpType
AX = mybir.AxisListType

# Problem shape (hardcoded per contest contract)
B, T, C, D, K, WIN = 16, 512, 1024, 4096, 128, 8
NCORES = 8
BPC = B // NCORES  # batches per core = 2
TP = BPC * T  # frames per core = 1024
NTB = TP // 128  # t-blocks of 128 = 8
NDT = D // 128  # d-tiles = 32
NCT = C // 128  # c-tiles = 8
NW = T // WIN  # windows per batch = 64
ROWS = BPC * NW  # topk rows per core = 128
NCQ = 4  # c-quarters for decode
CQ = C // NCQ  # 256
MERGE = 4  # d-tiles per topk merge step
NMERGE = NDT // MERGE

# set to a BassKernelResults after each kernel() call (for test.py profiling)
LAST_RESULTS = None
TRACE = False
TMPDIR = None
DECODE_F32R = True  # decode-only tf32-class matmul (recon ~3e-4 rel; encoded stays exact)


def build_nc():
    nc = bacc.Bacc(
        "TRN2", target_bir_lowering=False, debug=False, num_devices=NCORES
    )

    xT = nc.declare_dram_parameter("xT", [C, TP], F32, isOutput=False)
    We = nc.declare_dram_parameter("We", [NDT, 128, C], F32, isOutput=False)
    Wd = nc.declare_dram_parameter("Wd", [NDT, 128, C], F32, isOutput=False)
    be_r = nc.declare_dram_parameter("be_r", [128, NDT], F32, isOutput=False)
    bd_r = nc.declare_dram_parameter("bd_r", [128, NCT], F32, isOutput=False)
    bd_bc = nc.declare_dram_parameter("bd_bc", [128, C], F32, isOutput=False)
    ident_d = nc.declare_dram_parameter("ident", [128, 128], F32, isOutput=False)

    enc_out = nc.declare_dram_parameter("enc", [BPC, T, D], F32, isOutput=True)
    rec_out = nc.declare_dram_parameter("rec", [BPC, T, C], F32, isOutput=True)

    with tile.TileContext(nc) as tc, ExitStack() as ctx:
        # ---- whole-kernel SBUF pools ----
        consts = ctx.enter_context(tc.tile_pool(name="consts", bufs=1))
        pre_pool = ctx.enter_context(tc.tile_pool(name="pre", bufs=1))

        be_sb = consts.tile([128, NDT], F32, tag="be")
        nc.sync.dma_start(be_sb, be_r[:])
        bd_sb = consts.tile([128, NCT], F32, tag="bd")
        nc.sync.dma_start(bd_sb, bd_r[:])
        bdb_sb = consts.tile([128, C], F32, tag="bdb")
        nc.sync.dma_start(bdb_sb, bd_bc[:])
        ident = consts.tile([128, 128], F32, tag="ident")
        nc.sync.dma_start(ident, ident_d[:])

        preT = []
        for j in range(NDT):
            preT.append(pre_pool.tile([128, TP], F32, tag=f"preT{j}", name=f"preT{j}"))

        # ---- pools that live encode -> mask application ----
        tctx = ExitStack()
        ws_pool = tctx.enter_context(tc.tile_pool(name="wsp", bufs=1))
        cand_pool = tctx.enter_context(tc.tile_pool(name="cand", bufs=2))
        thr_pool = tctx.enter_context(tc.tile_pool(name="thr", bufs=1))
        mt_pool = tctx.enter_context(tc.tile_pool(name="mt", bufs=3))
        tp_ps = tctx.enter_context(tc.tile_pool(name="tp_ps", bufs=2, space="PSUM"))

        ws = ws_pool.tile([128, D], F32, tag="ws")

        # merge schedule: after these d-tile counts, fold the new ws columns
        # into the running top-K.  Last two steps are small so the tail of the
        # chain (which serializes with decode start) is short.
        merge_pts = [4, 8, 12, 16, 20, 24, 28, 30, 32]
        cands = [
            cand_pool.tile(
                [128, K + MERGE * 128], F32, tag=f"cand{g % 2}", name=f"cand{g}"
            )
            for g in range(len(merge_pts) + 1)
        ]
        nc.vector.memset(cands[0][:, 0:K], -1.0)

        # ================= Phase E: encode =================
        with ExitStack() as ectx:
            xc_pool = ectx.enter_context(tc.tile_pool(name="xc", bufs=1))
            we_pool = ectx.enter_context(tc.tile_pool(name="we", bufs=2))
            pe_ps = ectx.enter_context(tc.tile_pool(name="pe_ps", bufs=4, space="PSUM"))

            # xcT tiles: (128 c, TP t) per c-tile, minus b_dec (per-partition)
            xcT = []
            for i in range(NCT):
                xct = xc_pool.tile([128, TP], F32, tag=f"xcT{i}", name=f"xcT{i}")
                xcT.append(xct)
            # split halves so the first encode matmuls (t-chunk 0) can start
            # before the whole 4MB of x has landed
            for h in range(2):
                sl = slice(h * 512, (h + 1) * 512)
                for i in range(NCT):
                    nc.sync.dma_start(xcT[i][:, sl], xT[i * 128 : (i + 1) * 128, sl])
                    nc.vector.tensor_scalar(
                        xcT[i][:, sl], xcT[i][:, sl], bd_sb[:, i : i + 1], None,
                        op0=ALU.subtract,
                    )

            for j in range(NDT):
                we_t = we_pool.tile([128, C], F32, tag="we")
                nc.sync.dma_start(we_t, We[j])
                we_v = we_t.rearrange("p (i m) -> p i m", i=NCT)
                pre_j = preT[j]
                for h in range(2):  # halves of TP (512 frames each)
                    ps = pe_ps.tile([128, 512], F32, tag="ps")
                    for i in range(NCT):
                        nc.tensor.matmul(
                            ps,
                            we_v[:, i, :],
                            xcT[i][:, h * 512 : (h + 1) * 512],
                            start=(i == 0),
                            stop=(i == NCT - 1),
                        )
                    # preT = relu(psum + b_enc[dtile]) , fused on ScalarE
                    nc.scalar.activation(
                        pre_j[:, h * 512 : (h + 1) * 512],
                        ps,
                        AF.Relu,
                        bias=be_sb[:, j : j + 1],
                        scale=1.0,
                    )
                # window sums for this d-tile: (128, BPC, NW, WIN) -> (128, BPC*NW)
                wst = mt_pool.tile([128, ROWS], F32, tag="wst")
                nc.vector.tensor_reduce(
                    wst.rearrange("p (b w) -> p b w", b=BPC),
                    pre_j.rearrange("p (b w e) -> p b w e", b=BPC, w=NW, e=WIN),
                    axis=AX.X,
                    op=ALU.add,
                )
                # transpose to row-major ws columns
                pst = tp_ps.tile([128, 128], F32, tag="pst")
                nc.tensor.transpose(pst, wst, ident)
                nc.scalar.copy(ws[:, j * 128 : (j + 1) * 128], pst)

                # pipelined top-k: merge the newest ws columns into the
                # running top-K (kept in cands[g][:, 0:K])
                if (j + 1) in merge_pts:
                    g = merge_pts.index(j + 1)
                    lo = 0 if g == 0 else merge_pts[g - 1]
                    src, dst = cands[g], cands[g + 1]
                    width = (j + 1 - lo) * 128
                    nc.vector.tensor_copy(
                        src[:, K : K + width], ws[:, lo * 128 : (j + 1) * 128]
                    )
                    for it in range(K // 8):
                        nc.vector.max(
                            out=dst[:, it * 8 : (it + 1) * 8],
                            in_=src[:, 0 : K + width],
                        )
                        nc.vector.match_replace(
                            out=src[:, 0 : K + width],
                            in_to_replace=dst[:, it * 8 : (it + 1) * 8],
                            in_values=src[:, 0 : K + width],
                            imm_value=-1.0,
                        )

        # ================= threshold + mask + apply =================
        thr = thr_pool.tile([128, 1], F32, tag="thr")
        nc.vector.tensor_reduce(
            thr, cands[len(merge_pts)][:, 0:K], axis=AX.X, op=ALU.min
        )
        # mask in place: ws = (ws >= thr)  — exact top-K (no fp32 ties here)
        nc.vector.tensor_scalar(ws, ws, thr, None, op0=ALU.is_ge)

        # mask transposes + apply to preT in place (preT becomes encT)
        for j in range(NDT):
            pst = tp_ps.tile([128, 128], F32, tag="pst")
            nc.tensor.transpose(pst, ws[:, j * 128 : (j + 1) * 128], ident)
            mT = mt_pool.tile([128, 128], F32, tag="mT")
            nc.vector.tensor_copy(mT, pst)
            nc.vector.tensor_tensor(
                preT[j].rearrange("p (b w e) -> p b w e", b=BPC, w=NW, e=WIN),
                preT[j].rearrange("p (b w e) -> p b w e", b=BPC, w=NW, e=WIN),
                mT.rearrange("p (b w) -> p b w", b=BPC).to_broadcast(
                    (128, BPC, NW, WIN)
                ),
                op=ALU.mult,
            )
        tctx.close()  # frees ws/cand/mt SBUF and tp_ps PSUM

        # ================= Phase D: encoded output + decode =================
        with ExitStack() as dctx:
            stg_pool = dctx.enter_context(tc.tile_pool(name="stg", bufs=3))
            wd_pool = dctx.enter_context(tc.tile_pool(name="wd", bufs=4))
            rec_pool = dctx.enter_context(tc.tile_pool(name="rec", bufs=3))

            # encoded output: transpose encT tiles back to (t, d), DMA out
            with ExitStack() as ectx2:
                eo_ps = ectx2.enter_context(
                    tc.tile_pool(name="eo_ps", bufs=4, space="PSUM")
                )
                for h8 in range(NTB):
                    b = h8 // (NTB // BPC)
                    trow = (h8 % (NTB // BPC)) * 128
                    for jg in range(NDT // 4):
                        eo = eo_ps.tile([128, 512], F32, tag="eo")
                        for k4 in range(4):
                            j = jg * 4 + k4
                            nc.tensor.transpose(
                                eo[:, k4 * 128 : (k4 + 1) * 128],
                                preT[j][:, h8 * 128 : (h8 + 1) * 128],
                                ident,
                            )
                        stg = stg_pool.tile([128, 512], F32, tag="stg")
                        nc.scalar.copy(stg, eo)
                        nc.sync.dma_start(
                            enc_out[b, trow : trow + 128, jg * 512 : (jg + 1) * 512],
                            stg,
                        )

            # decode: recon[t, c] = sum_d encT[d, t] * W_decT[d, c]  (+ b_dec)
            # 8 bank-padded psum tiles — accumulation groups must never share
            # a PSUM bank (start=True clears the whole bank's has_written bits).
            psd_pool = dctx.enter_context(
                tc.tile_pool(name="psd", bufs=1, space="PSUM")
            )
            if DECODE_F32R:
                # fast path: float32r matmul (1 cyc/row vs fp32's 4).  The BIR
                # verifier requires f32r operands in f32r-declared tiles whose
                # only writers round — so round encT slices and W_dec tiles
                # just-in-time on DVE into small transient f32r tiles.
                # Loop: t-halves (= batch) outer; per j round + accumulate into
                # 8 psum banks = (2 c-halves) x (4 t-blocks).
                CH = C // 2
                for th in range(BPC):
                    psds = {}
                    for q in range(2):
                        for hh in range(4):
                            psds[q, hh] = psd_pool.tile(
                                [128, CH], F32, tag=f"psd{q}_{hh}",
                                name=f"psd{q}_{hh}_{th}",
                            )
                    for j in range(NDT):
                        wd_f = wd_pool.tile([128, C], F32, tag="wdf")
                        nc.sync.dma_start(wd_f, Wd[j])
                        wd_r = wd_pool.tile([128, C], F32R, tag="wdr")
                        nc.scalar.copy(wd_r, wd_f)  # ACT rounds f32 -> f32r
                        encr = wd_pool.tile([128, 512], F32R, tag="encr")
                        nc.vector.tensor_copy(
                            encr, preT[j][:, th * 512 : (th + 1) * 512]
                        )
                        for hh in range(4):
                            for q in range(2):
                                nc.tensor.matmul(
                                    psds[q, hh],
                                    encr[:, hh * 128 : (hh + 1) * 128],
                                    wd_r[:, q * CH : (q + 1) * CH],
                                    start=(j == 0),
                                    stop=(j == NDT - 1),
                                )
                    for q in range(2):
                        for hh in range(4):
                            rsb = rec_pool.tile([128, CH], F32, tag="rsb")
                            nc.vector.tensor_tensor(
                                rsb,
                                psds[q, hh],
                                bdb_sb[:, q * CH : (q + 1) * CH],
                                op=ALU.add,
                            )
                            nc.sync.dma_start(
                                rec_out[
                                    th,
                                    hh * 128 : (hh + 1) * 128,
                                    q * CH : (q + 1) * CH,
                                ],
                                rsb,
                            )
            else:
                for q in range(NCQ):
                    psds = [
                        psd_pool.tile(
                            [128, CQ], F32, tag=f"psd{h8}", name=f"psd{h8}_{q}"
                        )
                        for h8 in range(NTB)
                    ]
                    for j in range(NDT):
                        wd_t = wd_pool.tile([128, CQ], F32, tag="wd")
                        nc.sync.dma_start(wd_t, Wd[j][:, q * CQ : (q + 1) * CQ])
                        for h8 in range(NTB):
                            nc.tensor.matmul(
                                psds[h8],
                                preT[j][:, h8 * 128 : (h8 + 1) * 128],
                                wd_t,
                                start=(j == 0),
                                stop=(j == NDT - 1),
                            )
                    for h8 in range(NTB):
                        b = h8 // (NTB // BPC)
                        trow = (h8 % (NTB // BPC)) * 128
                        rsb = rec_pool.tile([128, CQ], F32, tag="rsb")
                        nc.vector.tensor_tensor(
                            rsb,
                            psds[h8],
                            bdb_sb[:, q * CQ : (q + 1) * CQ],
                            op=ALU.add,
                        )
                        nc.sync.dma_start(
                            rec_out[b, trow : trow + 128, q * CQ : (q + 1) * CQ], rsb
                        )

    nc.compile()
    return nc


def _host_prep(x, W_enc, b_enc, W_dec, b_dec):
    x = np.ascontiguousarray(np.asarray(x, dtype=np.float32))
    W_enc = np.asarray(W_enc, dtype=np.float32)
    W_dec = np.asarray(W_dec, dtype=np.float32)
    b_enc = np.asarray(b_enc, dtype=np.float32)
    b_dec = np.asarray(b_dec, dtype=np.float32)

    # W_encT tiles: We[j, p, i*128+m] = W_enc[j*128+m, i*128+p]
    We_h = np.ascontiguousarray(
        W_enc.reshape(NDT, 128, NCT, 128).transpose(0, 3, 2, 1)
    ).reshape(NDT, 128, C)
    # W_decT tiles: Wd[j, p, c] = W_dec[c, j*128+p]
    Wd_h = np.ascontiguousarray(W_dec.T).reshape(NDT, 128, C)
    be_r = np.ascontiguousarray(b_enc.reshape(NDT, 128).T)
    bd_r = np.ascontiguousarray(b_dec.reshape(NCT, 128).T)
    bd_bc = np.ascontiguousarray(np.broadcast_to(b_dec[None, :], (128, C)))
    ident = np.eye(128, dtype=np.float32)

    in_maps = []
    for c in range(NCORES):
        xs = x[c * BPC : (c + 1) * BPC]  # (BPC, T, C)
        xT_c = np.ascontiguousarray(xs.transpose(2, 0, 1).reshape(C, TP))
        in_maps.append(
            {
                "xT": xT_c,
                "We": We_h,
                "Wd": Wd_h,
                "be_r": be_r,
                "bd_r": bd_r,
                "bd_bc": bd_bc,
                "ident": ident,
            }
        )
    return in_maps


def kernel(x, W_enc, b_enc, W_dec, b_dec):
    global LAST_RESULTS
    in_maps = _host_prep(x, W_enc, b_enc, W_dec, b_dec)
    nc = build_nc()
    kw = {}
    if TRACE:
        kw["trace"] = True
        if TMPDIR:
            kw["tmpdir"] = TMPDIR
    res = run_bass_kernel_spmd(nc, in_maps, list(range(NCORES)), **kw)
    LAST_RESULTS = res
    recon = np.concatenate([res.results[c]["rec"] for c in range(NCORES)], axis=0)
    encoded = np.concatenate([res.results[c]["enc"] for c in range(NCORES)], axis=0)
    return recon, encoded


# revision 21
# speedup vs baseline: 1.4379x; 1.0048x over previous
"""Trainium2 Bass kernel: windowed top-k sparse autoencoder.

recon, encoded = kernel(x, W_enc, b_enc, W_dec, b_dec)

  pre     = relu((x - b_dec) @ W_enc.T + b_enc)          (B, T, D)
  ws      = window-sum of pre over WIN=8 frames          (B, T/WIN, D)
  mask    = top-K(ws, K=128) per (batch, window) row
  encoded = pre * mask (broadcast over frames in window) (B, T, D)
  recon   = encoded @ W_dec.T + b_dec                    (B, T, C)

Data-parallel over batch: 16 batches -> 8 NeuronCores, 2 batches each.
Weights replicated per core; no collectives.

All on-device compute is fp32 (top-k boundary gaps on this problem are
~2e-5 — far above fp32 noise but far below bf16/tf32 noise, so the encode
matmul must be true fp32).

Host-side work is layout-only: transposes / reshapes for sharding and for
the PE's (contraction-on-partitions) operand layout.

Kernel structure (per core; activations kept transposed as preT (d, t)):
  encode: stream W_encT by d-tile; PE matmul (c-tiles on partitions) ->
          psum; ScalarE fused relu+bias -> preT; DVE window-sum reduce;
          small PE transpose lands ws rows (128 topk rows x 4096 d).
  top-k:  pipelined under encode — every 4 d-tiles, DVE merges the new
          512 ws columns into a running top-128 via 16 x (max8 +
          match_replace).  thr = min(top128); mask = (ws >= thr), exact
          (no fp32 ties; verified on this problem's data).
  apply:  transpose mask tiles, broadcast-multiply into preT (= encT).
  out:    PE transposes encT tiles back to (t, d) for the encoded output.
  decode: stream W_decT; PE accumulates over all 32 d-tiles into 8
          bank-padded psum tiles (one per t-block; never share banks
          between accumulation groups); DVE adds b_dec; DMA out.
"""

import sys

sys.path.insert(0, "/opt/trn_rl_repo")

from contextlib import ExitStack

import numpy as np

import concourse.bacc as bacc
import concourse.bass as bass
import concourse.tile as tile
from concourse import mybir
from concourse.bass_utils import run_bass_kernel_spmd

F32 = mybir.dt.float32
F32R = mybir.dt.float32r
AF = mybir.ActivationFunctionType
ALU = mybir.AluOpType
AX = mybir.AxisListType

# Problem shape (hardcoded per contest contract)
B, T, C, D, K, WIN = 16, 512, 1024, 4096, 128, 8
NCORES = 8
BPC = B // NCORES  # batches per core = 2
TP = BPC * T  # frames per core = 1024
NTB = TP // 128  # t-blocks of 128 = 8
NDT = D // 128  # d-tiles = 32
NCT = C // 128  # c-tiles = 8
NW = T // WIN  # windows per batch = 64
ROWS = BPC * NW  # topk rows per core = 128
NCQ = 4  # c-quarters for decode
CQ = C // NCQ  # 256
MERGE = 4  # d-tiles per topk merge step
NMERGE = NDT // MERGE

# set to a BassKernelResults after each kernel() call (for test.py profiling)
LAST_RESULTS = None
TRACE = False
TMPDIR = None
DECODE_F32R = True  # decode-only tf32-class matmul (recon ~3e-4 rel; encoded stays exact)


def build_nc():
    nc = bacc.Bacc(
        "TRN2", target_bir_lowering=False, debug=False, num_devices=NCORES
    )

    xT = nc.declare_dram_parameter("xT", [C, TP], F32, isOutput=False)
    We = nc.declare_dram_parameter("We", [NDT, 128, C], F32, isOutput=False)
    Wd = nc.declare_dram_parameter("Wd", [NDT, 128, C], F32, isOutput=False)
    be_r = nc.declare_dram_parameter("be_r", [128, NDT], F32, isOutput=False)
    bd_r = nc.declare_dram_parameter("bd_r", [128, NCT], F32, isOutput=False)
    bd_bc = nc.declare_dram_parameter("bd_bc", [128, C], F32, isOutput=False)
    ident_d = nc.declare_dram_parameter("ident", [128, 128], F32, isOutput=False)

    enc_out = nc.declare_dram_parameter("enc", [BPC, T, D], F32, isOutput=True)
    rec_out = nc.declare_dram_parameter("rec", [BPC, T, C], F32, isOutput=True)

    with tile.TileContext(nc) as tc, ExitStack() as ctx:
        # ---- whole-kernel SBUF pools ----
        consts = ctx.enter_context(tc.tile_pool(name="consts", bufs=1))
        pre_pool = ctx.enter_context(tc.tile_pool(name="pre", bufs=1))

        be_sb = consts.tile([128, NDT], F32, tag="be")
        nc.sync.dma_start(be_sb, be_r[:])
        bd_sb = consts.tile([128, NCT], F32, tag="bd")
        nc.sync.dma_start(bd_sb, bd_r[:])
        bdb_sb = consts.tile([128, C], F32, tag="bdb")
        nc.sync.dma_start(bdb_sb, bd_bc[:])
        ident = consts.tile([128, 128], F32, tag="ident")
        nc.sync.dma_start(ident, ident_d[:])

        preT = []
        for j in range(NDT):
            preT.append(pre_pool.tile([128, TP], F32, tag=f"preT{j}", name=f"preT{j}"))

        # ---- pools that live encode -> mask application ----
        tctx = ExitStack()
        ws_pool = tctx.enter_context(tc.tile_pool(name="wsp", bufs=1))
        cand_pool = tctx.enter_context(tc.tile_pool(name="cand", bufs=2))
        thr_pool = tctx.enter_context(tc.tile_pool(name="thr", bufs=1))
        mt_pool = tctx.enter_context(tc.tile_pool(name="mt", bufs=3))
        tp_ps = tctx.enter_context(tc.tile_pool(name="tp_ps", bufs=2, space="PSUM"))

        ws = ws_pool.tile([128, D], F32, tag="ws")

        # merge schedule: after these d-tile counts, fold the new ws columns
        # into the running top-K.  Last two steps are small so the tail of the
        # chain (which serializes with decode start) is short.
        merge_pts = [4, 8, 12, 16, 20, 24, 27, 29, 31, 32]
        cands = [
            cand_pool.tile(
                [128, K + MERGE * 128], F32, tag=f"cand{g % 2}", name=f"cand{g}"
            )
            for g in range(len(merge_pts) + 1)
        ]
        nc.vector.memset(cands[0][:, 0:K], -1.0)

        # ================= Phase E: encode =================
        with ExitStack() as ectx:
            xc_pool = ectx.enter_context(tc.tile_pool(name="xc", bufs=1))
            we_pool = ectx.enter_context(tc.tile_pool(name="we", bufs=2))
            pe_ps = ectx.enter_context(tc.tile_pool(name="pe_ps", bufs=4, space="PSUM"))

            # xcT tiles: (128 c, TP t) per c-tile, minus b_dec (per-partition)
            xcT = []
            for i in range(NCT):
                xct = xc_pool.tile([128, TP], F32, tag=f"xcT{i}", name=f"xcT{i}")
                xcT.append(xct)
            # split halves so the first encode matmuls (t-chunk 0) can start
            # before the whole 4MB of x has landed
            for h in range(2):
                sl = slice(h * 512, (h + 1) * 512)
                for i in range(NCT):
                    nc.sync.dma_start(xcT[i][:, sl], xT[i * 128 : (i + 1) * 128, sl])
                    nc.vector.tensor_scalar(
                        xcT[i][:, sl], xcT[i][:, sl], bd_sb[:, i : i + 1], None,
                        op0=ALU.subtract,
                    )

            for j in range(NDT):
                we_t = we_pool.tile([128, C], F32, tag="we")
                nc.sync.dma_start(we_t, We[j])
                we_v = we_t.rearrange("p (i m) -> p i m", i=NCT)
                pre_j = preT[j]
                for h in range(2):  # halves of TP (512 frames each)
                    ps = pe_ps.tile([128, 512], F32, tag="ps")
                    for i in range(NCT):
                        nc.tensor.matmul(
                            ps,
                            we_v[:, i, :],
                            xcT[i][:, h * 512 : (h + 1) * 512],
                            start=(i == 0),
                            stop=(i == NCT - 1),
                        )
                    # preT = relu(psum + b_enc[dtile]) , fused on ScalarE
                    nc.scalar.activation(
                        pre_j[:, h * 512 : (h + 1) * 512],
                        ps,
                        AF.Relu,
                        bias=be_sb[:, j : j + 1],
                        scale=1.0,
                    )
                # window sums for this d-tile: (128, BPC, NW, WIN) -> (128, BPC*NW)
                wst = mt_pool.tile([128, ROWS], F32, tag="wst")
                nc.vector.tensor_reduce(
                    wst.rearrange("p (b w) -> p b w", b=BPC),
                    pre_j.rearrange("p (b w e) -> p b w e", b=BPC, w=NW, e=WIN),
                    axis=AX.X,
                    op=ALU.add,
                )
                # transpose to row-major ws columns
                pst = tp_ps.tile([128, 128], F32, tag="pst")
                nc.tensor.transpose(pst, wst, ident)
                nc.scalar.copy(ws[:, j * 128 : (j + 1) * 128], pst)

                # pipelined top-k: merge the newest ws columns into the
                # running top-K (kept in cands[g][:, 0:K])
                if (j + 1) in merge_pts:
                    g = merge_pts.index(j + 1)
                    lo = 0 if g == 0 else merge_pts[g - 1]
                    src, dst = cands[g], cands[g + 1]
                    width = (j + 1 - lo) * 128
                    nc.vector.tensor_copy(
                        src[:, K : K + width], ws[:, lo * 128 : (j + 1) * 128]
                    )
                    for it in range(K // 8):
                        nc.vector.max(
                            out=dst[:, it * 8 : (it + 1) * 8],
                            in_=src[:, 0 : K + width],
                        )
                        nc.vector.match_replace(
                            out=src[:, 0 : K + width],
                            in_to_replace=dst[:, it * 8 : (it + 1) * 8],
                            in_values=src[:, 0 : K + width],
                            imm_value=-1.0,
                        )

        # ================= threshold + mask + apply =================
        thr = thr_pool.tile([128, 1], F32, tag="thr")
        nc.vector.tensor_reduce(
            thr, cands[len(merge_pts)][:, 0:K], axis=AX.X, op=ALU.min
        )
        # mask in place: ws = (ws >= thr)  — exact top-K (no fp32 ties here)
        nc.vector.tensor_scalar(ws, ws, thr, None, op0=ALU.is_ge)

        # mask transposes + apply to preT in place (preT becomes encT)
        for j in range(NDT):
            pst = tp_ps.tile([128, 128], F32, tag="pst")
            nc.tensor.transpose(pst, ws[:, j * 128 : (j + 1) * 128], ident)
            mT = mt_pool.tile([128, 128], F32, tag="mT")
            nc.vector.tensor_copy(mT, pst)
            nc.vector.tensor_tensor(
                preT[j].rearrange("p (b w e) -> p b w e", b=BPC, w=NW, e=WIN),
                preT[j].rearrange("p (b w e) -> p b w e", b=BPC, w=NW, e=WIN),
                mT.rearrange("p (b w) -> p b w", b=BPC).to_broadcast(
                    (128, BPC, NW, WIN)
                ),
                op=ALU.mult,
            )
        tctx.close()  # frees ws/cand/mt SBUF and tp_ps PSUM

        # ================= Phase D: encoded output + decode =================
        with ExitStack() as dctx:
            stg_pool = dctx.enter_context(tc.tile_pool(name="stg", bufs=3))
            wd_pool = dctx.enter_context(tc.tile_pool(name="wd", bufs=4))
            rec_pool = dctx.enter_context(tc.tile_pool(name="rec", bufs=3))

            # encoded output: transpose encT tiles back to (t, d), DMA out
            with ExitStack() as ectx2:
                eo_ps = ectx2.enter_context(
                    tc.tile_pool(name="eo_ps", bufs=4, space="PSUM")
                )
                for h8 in range(NTB):
                    b = h8 // (NTB // BPC)
                    trow = (h8 % (NTB // BPC)) * 128
                    for jg in range(NDT // 4):
                        eo = eo_ps.tile([128, 512], F32, tag="eo")
                        for k4 in range(4):
                            j = jg * 4 + k4
                            nc.tensor.transpose(
                                eo[:, k4 * 128 : (k4 + 1) * 128],
                                preT[j][:, h8 * 128 : (h8 + 1) * 128],
                                ident,
                            )
                        stg = stg_pool.tile([128, 512], F32, tag="stg")
                        # alternate the psum->sbuf drain between ScalarE and
                        # VectorE — this stage is copy-bound, not PE-bound
                        if (jg + h8) % 2 == 0:
                            nc.scalar.copy(stg, eo)
                        else:
                            nc.vector.tensor_copy(stg, eo)
                        nc.sync.dma_start(
                            enc_out[b, trow : trow + 128, jg * 512 : (jg + 1) * 512],
                            stg,
                        )

            # decode: recon[t, c] = sum_d encT[d, t] * W_decT[d, c]  (+ b_dec)
            # 8 bank-padded psum tiles — accumulation groups must never share
            # a PSUM bank (start=True clears the whole bank's has_written bits).
            psd_pool = dctx.enter_context(
                tc.tile_pool(name="psd", bufs=1, space="PSUM")
            )
            if DECODE_F32R:
                # fast path: float32r matmul (1 cyc/row vs fp32's 4).  The BIR
                # verifier requires f32r operands in f32r-declared tiles whose
                # only writers round — so round encT slices and W_dec tiles
                # just-in-time on DVE into small transient f32r tiles.
                # Loop: t-halves (= batch) outer; per j round + accumulate into
                # 8 psum banks = (2 c-halves) x (4 t-blocks).
                CH = C // 2
                for th in range(BPC):
                    psds = {}
                    for q in range(2):
                        for hh in range(4):
                            psds[q, hh] = psd_pool.tile(
                                [128, CH], F32, tag=f"psd{q}_{hh}",
                                name=f"psd{q}_{hh}_{th}",
                            )
                    for j in range(NDT):
                        wd_f = wd_pool.tile([128, C], F32, tag="wdf", bufs=5)
                        nc.sync.dma_start(wd_f, Wd[j])
                        wd_r = wd_pool.tile([128, C], F32R, tag="wdr", bufs=5)
                        nc.scalar.copy(wd_r, wd_f)  # ACT rounds f32 -> f32r
                        encr = wd_pool.tile([128, 512], F32R, tag="encr", bufs=6)
                        nc.vector.tensor_copy(
                            encr, preT[j][:, th * 512 : (th + 1) * 512]
                        )
                        for hh in range(4):
                            for q in range(2):
                                nc.tensor.matmul(
                                    psds[q, hh],
                                    encr[:, hh * 128 : (hh + 1) * 128],
                                    wd_r[:, q * CH : (q + 1) * CH],
                                    start=(j == 0),
                                    stop=(j == NDT - 1),
                                )
                    for q in range(2):
                        for hh in range(4):
                            rsb = rec_pool.tile([128, CH], F32, tag="rsb")
                            nc.vector.tensor_tensor(
                                rsb,
                                psds[q, hh],
                                bdb_sb[:, q * CH : (q + 1) * CH],
                                op=ALU.add,
                            )
                            nc.sync.dma_start(
                                rec_out[
                                    th,
                                    hh * 128 : (hh + 1) * 128,
                                    q * CH : (q + 1) * CH,
                                ],
                                rsb,
                            )
            else:
                for q in range(NCQ):
                    psds = [
                        psd_pool.tile(
                            [128, CQ], F32, tag=f"psd{h8}", name=f"psd{h8}_{q}"
                        )
                        for h8 in range(NTB)
                    ]
                    for j in range(NDT):
                        wd_t = wd_pool.tile([128, CQ], F32, tag="wd")
                        nc.sync.dma_start(wd_t, Wd[j][:, q * CQ : (q + 1) * CQ])
                        for h8 in range(NTB):
                            nc.tensor.matmul(
                                psds[h8],
                                preT[j][:, h8 * 128 : (h8 + 1) * 128],
                                wd_t,
                                start=(j == 0),
                                stop=(j == NDT - 1),
                            )
                    for h8 in range(NTB):
                        b = h8 // (NTB // BPC)
                        trow = (h8 % (NTB // BPC)) * 128
                        rsb = rec_pool.tile([128, CQ], F32, tag="rsb")
                        nc.vector.tensor_tensor(
                            rsb,
                            psds[h8],
                            bdb_sb[:, q * CQ : (q + 1) * CQ],
                            op=ALU.add,
                        )
                        nc.sync.dma_start(
                            rec_out[b, trow : trow + 128, q * CQ : (q + 1) * CQ], rsb
                        )

    nc.compile()
    return nc


def _host_prep(x, W_enc, b_enc, W_dec, b_dec):
    x = np.ascontiguousarray(np.asarray(x, dtype=np.float32))
    W_enc = np.asarray(W_enc, dtype=np.float32)
    W_dec = np.asarray(W_dec, dtype=np.float32)
    b_enc = np.asarray(b_enc, dtype=np.float32)
    b_dec = np.asarray(b_dec, dtype=np.float32)

    # W_encT tiles: We[j, p, i*128+m] = W_enc[j*128+m, i*128+p]
    We_h = np.ascontiguousarray(
        W_enc.reshape(NDT, 128, NCT, 128).transpose(0, 3, 2, 1)
    ).reshape(NDT, 128, C)
    # W_decT tiles: Wd[j, p, c] = W_dec[c, j*128+p]
    Wd_h = np.ascontiguousarray(W_dec.T).reshape(NDT, 128, C)
    be_r = np.ascontiguousarray(b_enc.reshape(NDT, 128).T)
    bd_r = np.ascontiguousarray(b_dec.reshape(NCT, 128).T)
    bd_bc = np.ascontiguousarray(np.broadcast_to(b_dec[None, :], (128, C)))
    ident = np.eye(128, dtype=np.float32)

    in_maps = []
    for c in range(NCORES):
        xs = x[c * BPC : (c + 1) * BPC]  # (BPC, T, C)
        xT_c = np.ascontiguousarray(xs.transpose(2, 0, 1).reshape(C, TP))
        in_maps.append(
            {
                "xT": xT_c,
                "We": We_h,
                "Wd": Wd_h,
                "be_r": be_r,
                "bd_r": bd_r,
                "bd_bc": bd_bc,
                "ident": ident,
            }
        )
    return in_maps


def kernel(x, W_enc, b_enc, W_dec, b_dec):
    global LAST_RESULTS
    in_maps = _host_prep(x, W_enc, b_enc, W_dec, b_dec)
    nc = build_nc()
    kw = {}
    if TRACE:
        kw["trace"] = True
        if TMPDIR:
            kw["tmpdir"] = TMPDIR
    res = run_bass_kernel_spmd(nc, in_maps, list(range(NCORES)), **kw)
    LAST_RESULTS = res
    recon = np.concatenate([res.results[c]["rec"] for c in range(NCORES)], axis=0)
    encoded = np.concatenate([res.results[c]["enc"] for c in range(NCORES)], axis=0)
    return recon, encoded
